# revision 29
# baseline (speedup 1.0000x reference)
"""Trainium2 Bass kernel for nn_BaseRuleLearner (pipelined compact design).

Math (per batch element b, reference semantics):
  UM[b,i,v,l]      = sum_e U[b,l,e]  * ru[i,v,e]
  BM[b,i,n,m,j,k]  = sum_e Bf[b,j,k,e] * rb[i,n,m,e]
  scores[b,i,p]    = sum_v UM[b,i,v,perm[p,v]]
                   + sum_{n,m} BM[b,i,n,m,perm[p,n],perm[p,m]]
  merged[b,i]      = min_p scores[b,i,p]
  out[b,:]         = softmax_i(merged) @ one_hot([0,0,1,1])

Design (pure data parallel over B across 8 cores, BC=512 b/core):

Stage-1 (36 matmuls, 12 psum banks x 3 tile_position slots {0,32,64}):
  offdiag slot jp: psum rows ud*4+i (24) = wb.T @ ab[:, jp cols]
    (pair-packed columns [Bf j,k ; Bf k,j], weights per (u,d,i))
  unary slot l:    psum rows v*4+i (12) = wu.T @ au[:, l cols]
    (columns [U l ; Bf l,l], weights [ru i,v ; rb i,v,v])
  Banks: U0=[l0..l2] U1=[l3..l5] U2=[l6,l7,jp27] O0..O8=[jp0..jp26].

Assembly through DRAM (i moves from psum rows to qt columns; engine
copies cannot cross partitions; SBUF-source multi-partition-dim DMAs
scramble):  evac casts psum[96,512] -> sg bf16; hop1 (trivial APs)
sg -> scr[g, p*512+b]; hop2 re-views scr affinely as rows (g,slot,sub)
x cols (i,b), skipping pad rows, into compact qt tiles:
  qtA [66, 2048] = U0,U1,U2,O0,O1   qtB [126, 2048] = O2..O8
RAW through DRAM is not dep-tracked -> set_after_insts.  Per-group
scratch tensors (a0,a1,b1,b2) avoid WAW serialization.  Queues: inputs
+ a1/b2 hop chains + per-bt out on sync; a0/b1 hop chains on scalar
(per-hw-engine ring FIFO keeps same-queue order sane; the tail b2
chain rides the sync queue, idle after the input stream).

Stage-2 (32 matmuls): per (bt,i): psum[128,336] = qtA_slice.T @ GA
(start) + qtB_slice.T @ GB (stop); G are 0/1 built to match qt rows.
DVE min-reduce over 336 perms -> merged.  Softmax without
max-subtraction (|logits| < ~10, exp safe in fp32): exp+accum (ACT),
reciprocal, pair-add, scale.  Out [128, 4*4] fp32, host reorders to
[512, 4].

Pipelining: DMA order wa(weights+G+unary), ab1(jp27,jp0-5),
ab2(jp6-17), ab3(jp18-26); evacs alternate vector/scalar; chunk-A hops
run mid-kernel; 6 early chunk-A stage-2 matmuls fill the PE while ab3
streams; closing interleaves mmB/mmA under psum pressure (pss bufs=6,
psb bufs=2 -- NOTE: other psb/pss splits (3/5, 4/4) produced NaNs on
hardware, apparently a latent scheduling race exposed by timing shifts;
keep 2/6).

Min-reduces: tiles 12-15 (the critical tail) and even tiles reduce
direct on DVE from psum; odd tiles 1-11 are copied psum->bf16 by the
ACT engine first then bf16-reduced on DVE (the closing is otherwise
DVE-reduce-bound; the copy path has higher latency so it is kept off
the final tiles). Splitting each evac across both engines was tried
and is SLOWER (op-count overhead beats the width win).

Known profile (50-54us; +-3-4us thermal/throttle variance between
measurement windows): input stream ~7-24us at the ~400GB/s aggregate
cap (6.9MB total incl. 1.96MB hop round-trip), stage-1 trails to
~26-28us (psb rotation; psb/pss splits other than 2/6 NaN on hw),
closing matmuls to ~44us, ~1.3us preamble + ~8-9us fixed NEFF
semaphore-wipe epilogue (measured ~9.4us on a trivial 2-DMA kernel,
unavoidable from kernel structure).
"""

import itertools
import numpy as np

B, O, E = 4096, 8, 64
I, V = 4, 3
P = 336
N_CORES = 8
BC = B // N_CORES            # 512 batch per core
NP = 28
SGR = 88                     # sg rows carried per bank (3 slots, 24 used each)
SCRP = 96                    # scr row pitch (virtual; rows 88-95 never written)
KA, KB = 66, 126             # stage-2 k-chunk rows

_PERM = np.array(list(itertools.permutations(range(O), V)), dtype=np.int32)
_PAIRS = [(j, k) for j in range(O) for k in range(j + 1, O)]
_PIDX = np.full((O, O), -1, np.int32)
for _i, (_j, _k) in enumerate(_PAIRS):
    _PIDX[_j, _k] = _i
_PAIRS3 = [(0, 1), (0, 2), (1, 2)]

AB_ORDER = [27] + list(range(27))            # ab column order (jp27 first)
AB_POS = {jp: idx for idx, jp in enumerate(AB_ORDER)}
AB_CH = [7, 12, 9]                           # ab DMA chunks (jp counts)

BANKS = (
    [[('u', 0), ('u', 1), ('u', 2)],
     [('u', 3), ('u', 4), ('u', 5)],
     [('u', 6), ('u', 7), ('o', 27)]]
    + [[('o', 3 * g + 0), ('o', 3 * g + 1), ('o', 3 * g + 2)] for g in range(9)]
)
NB_A = 5                     # banks 0..4 -> chunk A (U0,U1,U2,O0,O1)

# hop2 geometry: (chunk, r0, g0, ng, s0, ns, u0, nu) — single source of
# truth for both the device DMAs and the host-side G row mapping.
# scratch groups: grp -> (first bank, n banks)
GRPS = {'a0': (0, 3), 'a1': (3, 2), 'b1a': (5, 2), 'b1b': (7, 2),
        'b2a': (9, 1), 'b2b': (10, 1), 'b2c': (11, 1)}
HOP2S = [
    ('A', 0, 'a0', 0, 3, 0, 3, 0, 3),   # U0,U1,U2 all slots, subs 0..2
    ('A', 27, 'a0', 2, 1, 2, 1, 3, 3),  # U2 slot2 (jp27) subs 3..5
    ('A', 30, 'a1', 0, 2, 0, 3, 0, 6),  # O0,O1
    ('B', 0, 'b1a', 0, 2, 0, 3, 0, 6),   # O2,O3
    ('B', 36, 'b1b', 0, 2, 0, 3, 0, 6),  # O4,O5
    ('B', 72, 'b2a', 0, 1, 0, 3, 0, 6),  # O6
    ('B', 90, 'b2b', 0, 1, 0, 3, 0, 6),  # O7
    ('B', 108, 'b2c', 0, 1, 0, 3, 0, 6),  # O8
]

WGC = 24 + 12 + 2 * P        # wg cols: wb, wu, GA, GB

_CACHED = {}


def _qt_row_index():
    """(kind, ident, sub) -> (which, row) per HOP2S order."""
    idx = {}
    for chunk, r0, grp, gl, ng, s0, ns, u0, nu in HOP2S:
        g0 = GRPS[grp][0] + gl
        r = r0
        for g in range(g0, g0 + ng):
            for s in range(s0, s0 + ns):
                kind, ident = BANKS[g][s]
                for u in range(u0, u0 + nu):
                    idx[(kind, ident, u)] = (chunk, r)
                    r += 1
    return idx


def _build_g():
    idx = _qt_row_index()
    ga = np.zeros((KA, P), np.float32)
    gb = np.zeros((KB, P), np.float32)

    def mark(key, p):
        chunk, r = idx[key]
        (ga if chunk == 'A' else gb)[r, p] = 1.0

    for p in range(P):
        perm = _PERM[p]
        for v in range(V):
            mark(('u', int(perm[v]), v), p)
        for u, (n, m) in enumerate(_PAIRS3):
            x, y = int(perm[n]), int(perm[m])
            j, k = min(x, y), max(x, y)
            d = int(x > y)
            mark(('o', int(_PIDX[j, k]), u * 2 + d), p)
    return ga, gb


def _build_weights(ru, rb):
    wb = np.zeros((128, 24), np.float32)
    for u, (n, m) in enumerate(_PAIRS3):
        for d in range(2):
            for i in range(I):
                col = (u * 2 + d) * 4 + i
                fst, snd = ((n, m), (m, n)) if d == 0 else ((m, n), (n, m))
                wb[0:64, col] = rb[i, fst[0], fst[1], :]
                wb[64:128, col] = rb[i, snd[0], snd[1], :]
    wu = np.zeros((128, 12), np.float32)
    for v in range(V):
        for i in range(I):
            wu[0:64, v * 4 + i] = ru[i, v, :]
            wu[64:128, v * 4 + i] = rb[i, v, v, :]
    return wb, wu


def _build_module():
    import concourse.tile as tile
    from concourse import bacc, mybir

    FP = mybir.dt.float32
    BF = mybir.dt.bfloat16
    MIN = mybir.AluOpType.min
    nc = bacc.Bacc("TRN2", target_bir_lowering=False, debug=False)

    wa = nc.dram_tensor("wa", [128, WGC + O * BC], BF, kind="ExternalInput")
    ab = nc.dram_tensor("ab", [128, NP * BC], BF, kind="ExternalInput")
    out = nc.dram_tensor("out", [128, 16], FP, kind="ExternalOutput")
    scrs = {
        k: nc.dram_tensor(f"scr_{k}", [n, SCRP * BC], BF, kind="Internal")
        for k, (_, n) in GRPS.items()
    }

    with tile.TileContext(nc) as tc:
        with (
            tc.tile_pool(name="wpool", bufs=1) as wpool,
            tc.tile_pool(name="mpool", bufs=1) as mpool,
            tc.tile_pool(name="psb", bufs=2, space="PSUM") as psb,
            tc.tile_pool(name="pss", bufs=6, space="PSUM") as pss,
        ):
            # ---- persistent tiles ----
            wa_sb = wpool.tile([128, WGC + O * BC], BF, tag="wa")
            wg_sb = wa_sb[:, 0:WGC]
            au_sb = wa_sb[:, WGC:]
            ab_sb = []
            off = 0
            for c, njp in enumerate(AB_CH):
                t = wpool.tile([128, njp * BC], BF, tag=f"ab{c}")
                ab_sb.append((t, off))
                off += njp
            sgs = {
                k: wpool.tile([SGR, n * BC], BF, tag=f"sg_{k}", name=f"sg_{k}")
                for k, (_, n) in GRPS.items()
            }
            qts = {
                'A': wpool.tile([KA, I * BC], BF, tag="qtA", name="qtA"),
                'B': wpool.tile([KB, I * BC], BF, tag="qtB", name="qtB"),
            }
            merged = mpool.tile([128, 16], FP, tag="mg")
            scb = mpool.tile([128, P], BF, tag="scb", bufs=2, name="scb")
            ex = mpool.tile([128, 16], FP, tag="ex")
            sm = mpool.tile([128, 4], FP, tag="sm")
            rc = mpool.tile([128, 4], FP, tag="rc")
            pa = mpool.tile([128, 8], FP, tag="pa")
            fin = mpool.tile([128, 16], FP, tag="fin")

            wb_sb = wg_sb[:, 0:24]
            wu_sb = wg_sb[:, 24:36]
            g_sb = {
                'A': wg_sb[0:KA, 36:36 + P],
                'B': wg_sb[0:KB, 36 + P:36 + 2 * P],
            }

            # ---- input DMAs (sync queue: strict ring order) ----
            nc.sync.dma_start(wa_sb[:], wa.ap()[:])
            off = 0
            for c, njp in enumerate(AB_CH):
                nc.sync.dma_start(
                    ab_sb[c][0][:], ab.ap()[:, off * BC:(off + njp) * BC]
                )
                off += njp
            nc.vector.memset(fin[:], 0.0)

            # ---- stage-1 helpers ----
            def bank_mms(g, pb):
                for s, (kind, ident) in enumerate(BANKS[g]):
                    if kind == 'u':
                        lhs = wu_sb
                        dat = au_sb[:, ident * BC:(ident + 1) * BC]
                        m = 12
                    else:
                        lhs = wb_sb
                        pos = AB_POS[ident]
                        for (tl, o0), njp in zip(ab_sb, AB_CH):
                            if pos < o0 + njp:
                                dat = tl[:, (pos - o0) * BC:(pos - o0 + 1) * BC]
                                break
                        m = 24
                    nc.tensor.matmul(
                        pb[32 * s:32 * s + m, :], lhs, dat,
                        start=True, stop=True,
                    )

            def evac(g, sgt, cg):
                pb = psb.tile([SGR, BC], FP, tag="pb")
                bank_mms(g, pb)
                dst = sgt[:, cg * BC:(cg + 1) * BC]
                if g % 2:
                    nc.scalar.activation(
                        dst, pb[:], mybir.ActivationFunctionType.Copy
                    )
                else:
                    nc.vector.tensor_copy(dst, pb[:])

            def hop1(grp, eng):
                ng = GRPS[grp][1]
                dst = scrs[grp].ap()[:, :].rearrange(
                    "g (p b) -> p g b", p=SCRP
                )[0:SGR]
                src = sgs[grp][:, 0:ng * BC].rearrange(
                    "p (g b) -> p g b", g=ng
                )
                return eng.dma_start(dst, src)

            def hop2(spec, h1, eng, split=None):
                chunk, r0, grp, gl, ng, s0, ns, u0, nu = spec
                qtc = qts[chunk]
                tc.dep_state.set_after_insts(qtc.tensor.name, h1.ins)
                src = scrs[grp].ap()[gl:gl + ng, :].rearrange(
                    "g (s u ib) -> g s u ib", s=3, u=8
                )[:, s0:s0 + ns, u0:u0 + nu, :]
                nr = ng * ns * nu
                kw = {"max_dma_last_dim": split} if split else {}
                eng.dma_start(qtc[r0:r0 + nr, :], src, **kw)

            # ---- stage-2 helpers ----
            sc_tiles = {}

            def mmA(t):
                bt, i = t // 4, t % 4
                sc = pss.tile([128, P], FP, tag="sc")
                sc_tiles[t] = sc
                c = i * BC + bt * 128
                nc.tensor.matmul(
                    sc[:], qts['A'][:, c:c + 128], g_sb['A'],
                    start=True, stop=False,
                )

            def softmax(bt):
                # exp on ACT without the accumulator read-back; the sum
                # runs on DVE so the recip chain stays on one engine
                nc.scalar.activation(
                    ex[:, 4 * bt:4 * bt + 4], merged[:, 4 * bt:4 * bt + 4],
                    mybir.ActivationFunctionType.Exp,
                )
                nc.vector.tensor_reduce(
                    sm[:, bt:bt + 1], ex[:, 4 * bt:4 * bt + 4],
                    axis=mybir.AxisListType.X, op=mybir.AluOpType.add,
                )
                nc.vector.reciprocal(rc[:, bt:bt + 1], sm[:, bt:bt + 1])
                e2 = ex[:, 4 * bt:4 * bt + 4].rearrange(
                    "p (a two) -> p a two", two=2
                )
                nc.vector.tensor_add(
                    pa[:, 2 * bt:2 * bt + 2], e2[:, :, 0], e2[:, :, 1]
                )
                nc.vector.tensor_scalar_mul(
                    fin[:, 4 * bt:4 * bt + 2], pa[:, 2 * bt:2 * bt + 2],
                    rc[:, bt:bt + 1],
                )
                nc.sync.dma_start(
                    out.ap()[:, 4 * bt:4 * bt + 4], fin[:, 4 * bt:4 * bt + 4]
                )

            def mmB(t):
                bt, i = t // 4, t % 4
                sc = sc_tiles.pop(t)
                c = i * BC + bt * 128
                nc.tensor.matmul(
                    sc[:], qts['B'][:, c:c + 128], g_sb['B'],
                    start=False, stop=True,
                )
                if t % 2 == 0 or t >= 12:
                    nc.vector.tensor_reduce(
                        merged[:, t:t + 1], sc[:],
                        axis=mybir.AxisListType.X, op=MIN,
                    )
                else:
                    # offload psum read to the ACT engine; bf16 reduce is
                    # ~2x faster on DVE and the rounding is within budget
                    nc.scalar.activation(
                        scb[:], sc[:], mybir.ActivationFunctionType.Copy
                    )
                    nc.vector.tensor_reduce(
                        merged[:, t:t + 1], scb[:],
                        axis=mybir.AxisListType.X, op=MIN,
                    )
                if i == 3:
                    softmax(bt)

            # ---- stage-1 + interleaved stage-2 ----
            for g in range(0, 3):                    # U0,U1,U2
                evac(g, sgs['a0'], g)
            h1a0 = hop1('a0', nc.scalar)
            hop2(HOP2S[0], h1a0, nc.scalar)
            hop2(HOP2S[1], h1a0, nc.scalar)

            for g in range(3, 5):                    # O0,O1
                evac(g, sgs['a1'], g - 3)
            h1a1 = hop1('a1', nc.sync)
            hop2(HOP2S[2], h1a1, nc.sync)

            for g in range(5, 7):                    # O2,O3 (ab2)
                evac(g, sgs['b1a'], g - 5)
            h = hop1('b1a', nc.scalar)
            hop2(HOP2S[3], h, nc.scalar)

            for g in range(7, 9):                    # O4,O5 (ab2)
                evac(g, sgs['b1b'], g - 7)
            h = hop1('b1b', nc.scalar)
            hop2(HOP2S[4], h, nc.scalar)

            for t in range(6):                       # early chunk-A mms
                mmA(t)

            # per-bank tail chains: each hop fires as soon as its bank's
            # evac lands instead of waiting for the whole group
            for k, (gi, grp) in enumerate(
                ((9, 'b2a'), (10, 'b2b'), (11, 'b2c'))
            ):
                evac(gi, sgs[grp], 0)
                h = hop1(grp, nc.sync)
                hop2(HOP2S[5 + k], h, nc.sync, split=512)

            # ---- closing ----
            for t in range(16):
                mmB(t)
                if t + 6 < 16:
                    mmA(t + 6)


    nc.compile()
    return nc


def _get_module():
    if "nc" not in _CACHED:
        _CACHED["nc"] = _build_module()
    return _CACHED["nc"]


def _host_inputs(unary_feats, binary_feats, rule_unary, rule_binary):
    import ml_dtypes

    bf16 = ml_dtypes.bfloat16
    uf = np.asarray(unary_feats, dtype=np.float32).astype(bf16)
    bf = np.asarray(binary_feats, dtype=np.float32).astype(bf16)
    ru = np.asarray(rule_unary, dtype=np.float32)
    rb = np.asarray(rule_binary, dtype=np.float32)

    wb, wu = _build_weights(ru, rb)
    ga, gb = _build_g()
    wgm = np.zeros((128, WGC), np.float32)
    wgm[:, 0:24] = wb
    wgm[:, 24:36] = wu
    wgm[0:KA, 36:36 + P] = ga
    wgm[0:KB, 36 + P:36 + 2 * P] = gb
    wgm = wgm.astype(bf16)

    J = np.array([p[0] for p in _PAIRS])
    K = np.array([p[1] for p in _PAIRS])
    ordr = np.array(AB_ORDER)
    dia = np.arange(O)
    in_maps = []
    for c in range(N_CORES):
        bfc = bf[c * BC:(c + 1) * BC]                  # [BC, O, O, E]
        x0 = bfc.transpose(1, 2, 3, 0)                 # [j, k, e, b]
        pair = np.concatenate([x0[J, K], x0[K, J]], axis=1)  # [28, 128, BC]
        abm = np.ascontiguousarray(
            pair[ordr].transpose(1, 0, 2)
        ).reshape(128, NP * BC)
        ufc = uf[c * BC:(c + 1) * BC]
        ut = ufc.transpose(1, 2, 0)
        dg = bfc[:, dia, dia, :].transpose(1, 2, 0)
        aum = np.ascontiguousarray(
            np.concatenate([ut, dg], axis=1).transpose(1, 0, 2)
        ).reshape(128, O * BC)
        in_maps.append({"ab": abm, "wa": np.concatenate([wgm, aum], axis=1)})
    return in_maps


TRACE = False  # set True (e.g. from test.py) to capture an NTFF profile


def kernel(unary_feats, binary_feats, rule_unary, rule_binary):
    from concourse.bass_utils import run_bass_kernel_spmd

    nc = _get_module()
    in_maps = _host_inputs(unary_feats, binary_feats, rule_unary, rule_binary)
    res = run_bass_kernel_spmd(
        nc, in_maps, core_ids=list(range(N_CORES)), trace=TRACE
    )
    _CACHED["last_results"] = res
    outs = []
    for c in range(N_CORES):
        o = res.results[c]["out"]                      # [128, 16]
        outs.append(
            np.ascontiguousarray(
                o.reshape(128, 4, 4).transpose(1, 0, 2)
            ).reshape(BC, 4)
        )
    return np.concatenate(outs, axis=0)


# revision 30
# speedup vs baseline: 1.0365x; 1.0365x over previous
"""Trainium2 Bass kernel for nn_BaseRuleLearner (pipelined compact design).

Math (per batch element b, reference semantics):
  UM[b,i,v,l]      = sum_e U[b,l,e]  * ru[i,v,e]
  BM[b,i,n,m,j,k]  = sum_e Bf[b,j,k,e] * rb[i,n,m,e]
  scores[b,i,p]    = sum_v UM[b,i,v,perm[p,v]]
                   + sum_{n,m} BM[b,i,n,m,perm[p,n],perm[p,m]]
  merged[b,i]      = min_p scores[b,i,p]
  out[b,:]         = softmax_i(merged) @ one_hot([0,0,1,1])

Design (pure data parallel over B across 8 cores, BC=512 b/core):

Stage-1 (36 matmuls, 12 psum banks x 3 tile_position slots {0,32,64}):
  offdiag slot jp: psum rows ud*4+i (24) = wb.T @ ab[:, jp cols]
    (pair-packed columns [Bf j,k ; Bf k,j], weights per (u,d,i))
  unary slot l:    psum rows v*4+i (12) = wu.T @ au[:, l cols]
    (columns [U l ; Bf l,l], weights [ru i,v ; rb i,v,v])
  Banks: U0=[l0..l2] U1=[l3..l5] U2=[l6,l7,jp27] O0..O8=[jp0..jp26].

Assembly through DRAM (i moves from psum rows to qt columns; engine
copies cannot cross partitions; SBUF-source multi-partition-dim DMAs
scramble):  evac casts psum[96,512] -> sg bf16; hop1 (trivial APs)
sg -> scr[g, p*512+b]; hop2 re-views scr affinely as rows (g,slot,sub)
x cols (i,b), skipping pad rows, into compact qt tiles:
  qtA [66, 2048] = U0,U1,U2,O0,O1   qtB [126, 2048] = O2..O8
RAW through DRAM is not dep-tracked -> set_after_insts.  Per-group
scratch tensors (a0,a1,b1,b2) avoid WAW serialization.  Queues: inputs
+ a1/b2 hop chains + per-bt out on sync; a0/b1 hop chains on scalar
(per-hw-engine ring FIFO keeps same-queue order sane; the tail b2
chain rides the sync queue, idle after the input stream).

Stage-2 (32 matmuls): per (bt,i): psum[128,336] = qtA_slice.T @ GA
(start) + qtB_slice.T @ GB (stop); G are 0/1 built to match qt rows.
DVE min-reduce over 336 perms -> merged.  Softmax without
max-subtraction (|logits| < ~10, exp safe in fp32): exp+accum (ACT),
reciprocal, pair-add, scale.  Out [128, 4*4] fp32, host reorders to
[512, 4].

Pipelining: DMA order wa(weights+G+unary), ab1(jp27,jp0-5),
ab2(jp6-17), ab3(jp18-26); evacs alternate vector/scalar; chunk-A hops
run mid-kernel; 6 early chunk-A stage-2 matmuls fill the PE while ab3
streams; closing interleaves mmB/mmA under psum pressure (pss bufs=6,
psb bufs=2 -- NOTE: other psb/pss splits (3/5, 4/4) produced NaNs on
hardware, apparently a latent scheduling race exposed by timing shifts;
keep 2/6).

Min-reduces: tiles 12-15 (the critical tail) and even tiles reduce
direct on DVE from psum; odd tiles 1-11 are copied psum->bf16 by the
ACT engine first then bf16-reduced on DVE (the closing is otherwise
DVE-reduce-bound; the copy path has higher latency so it is kept off
the final tiles). Splitting each evac across both engines was tried
and is SLOWER (op-count overhead beats the width win).

Known profile (50-54us; +-3-4us thermal/throttle variance between
measurement windows): input stream ~7-24us at the ~400GB/s aggregate
cap (6.9MB total incl. 1.96MB hop round-trip), stage-1 trails to
~26-28us (psb rotation; psb/pss splits other than 2/6 NaN on hw),
closing matmuls to ~44us, ~1.3us preamble + ~8-9us fixed NEFF
semaphore-wipe epilogue (measured ~9.4us on a trivial 2-DMA kernel,
unavoidable from kernel structure).
"""

import itertools
import numpy as np

B, O, E = 4096, 8, 64
I, V = 4, 3
P = 336
N_CORES = 8
BC = B // N_CORES            # 512 batch per core
NP = 28
SGR = 88                     # sg rows carried per bank (3 slots, 24 used each)
SCRP = 96                    # scr row pitch (virtual; rows 88-95 never written)
KA, KB = 66, 126             # stage-2 k-chunk rows

_PERM = np.array(list(itertools.permutations(range(O), V)), dtype=np.int32)
_PAIRS = [(j, k) for j in range(O) for k in range(j + 1, O)]
_PIDX = np.full((O, O), -1, np.int32)
for _i, (_j, _k) in enumerate(_PAIRS):
    _PIDX[_j, _k] = _i
_PAIRS3 = [(0, 1), (0, 2), (1, 2)]

AB_ORDER = [27] + list(range(27))            # ab column order (jp27 first)
AB_POS = {jp: idx for idx, jp in enumerate(AB_ORDER)}
AB_CH = [7, 12, 9]                           # ab DMA chunks (jp counts)

BANKS = (
    [[('u', 0), ('u', 1), ('u', 2)],
     [('u', 3), ('u', 4), ('u', 5)],
     [('u', 6), ('u', 7), ('o', 27)]]
    + [[('o', 3 * g + 0), ('o', 3 * g + 1), ('o', 3 * g + 2)] for g in range(9)]
)
NB_A = 5                     # banks 0..4 -> chunk A (U0,U1,U2,O0,O1)

# hop2 geometry: (chunk, r0, g0, ng, s0, ns, u0, nu) — single source of
# truth for both the device DMAs and the host-side G row mapping.
# scratch groups: grp -> (first bank, n banks)
GRPS = {'a0': (0, 3), 'a1': (3, 2), 'b1': (5, 4), 'b2': (9, 3)}
HOP2S = [
    ('A', 0, 'a0', 0, 3, 0, 3, 0, 3),   # U0,U1,U2 all slots, subs 0..2
    ('A', 27, 'a0', 2, 1, 2, 1, 3, 3),  # U2 slot2 (jp27) subs 3..5
    ('A', 30, 'a1', 0, 2, 0, 3, 0, 6),  # O0,O1
    ('B', 0, 'b1', 0, 4, 0, 3, 0, 6),   # O2..O5 (ab2 banks)
    ('B', 72, 'b2', 0, 3, 0, 3, 0, 6),  # O6..O8 (ab3 banks)
]

WGC = 24 + 12 + 2 * P        # wg cols: wb, wu, GA, GB

_CACHED = {}


def _qt_row_index():
    """(kind, ident, sub) -> (which, row) per HOP2S order."""
    idx = {}
    for chunk, r0, grp, gl, ng, s0, ns, u0, nu in HOP2S:
        g0 = GRPS[grp][0] + gl
        r = r0
        for g in range(g0, g0 + ng):
            for s in range(s0, s0 + ns):
                kind, ident = BANKS[g][s]
                for u in range(u0, u0 + nu):
                    idx[(kind, ident, u)] = (chunk, r)
                    r += 1
    return idx


def _build_g():
    idx = _qt_row_index()
    ga = np.zeros((KA, P), np.float32)
    gb = np.zeros((KB, P), np.float32)

    def mark(key, p):
        chunk, r = idx[key]
        (ga if chunk == 'A' else gb)[r, p] = 1.0

    for p in range(P):
        perm = _PERM[p]
        for v in range(V):
            mark(('u', int(perm[v]), v), p)
        for u, (n, m) in enumerate(_PAIRS3):
            x, y = int(perm[n]), int(perm[m])
            j, k = min(x, y), max(x, y)
            d = int(x > y)
            mark(('o', int(_PIDX[j, k]), u * 2 + d), p)
    return ga, gb


def _build_weights(ru, rb):
    wb = np.zeros((128, 24), np.float32)
    for u, (n, m) in enumerate(_PAIRS3):
        for d in range(2):
            for i in range(I):
                col = (u * 2 + d) * 4 + i
                fst, snd = ((n, m), (m, n)) if d == 0 else ((m, n), (n, m))
                wb[0:64, col] = rb[i, fst[0], fst[1], :]
                wb[64:128, col] = rb[i, snd[0], snd[1], :]
    wu = np.zeros((128, 12), np.float32)
    for v in range(V):
        for i in range(I):
            wu[0:64, v * 4 + i] = ru[i, v, :]
            wu[64:128, v * 4 + i] = rb[i, v, v, :]
    return wb, wu


def _build_module():
    import concourse.tile as tile
    from concourse import bacc, mybir

    FP = mybir.dt.float32
    BF = mybir.dt.bfloat16
    MIN = mybir.AluOpType.min
    nc = bacc.Bacc("TRN2", target_bir_lowering=False, debug=False)

    wa = nc.dram_tensor("wa", [128, WGC + O * BC], BF, kind="ExternalInput")
    ab = nc.dram_tensor("ab", [128, NP * BC], BF, kind="ExternalInput")
    out = nc.dram_tensor("out", [128, 16], FP, kind="ExternalOutput")
    scrs = {
        k: nc.dram_tensor(f"scr_{k}", [n, SCRP * BC], BF, kind="Internal")
        for k, (_, n) in GRPS.items()
    }

    with tile.TileContext(nc) as tc:
        with (
            tc.tile_pool(name="wpool", bufs=1) as wpool,
            tc.tile_pool(name="mpool", bufs=1) as mpool,
            tc.tile_pool(name="psb", bufs=2, space="PSUM") as psb,
            tc.tile_pool(name="pss", bufs=6, space="PSUM") as pss,
        ):
            # ---- persistent tiles ----
            wa_sb = wpool.tile([128, WGC + O * BC], BF, tag="wa")
            wg_sb = wa_sb[:, 0:WGC]
            au_sb = wa_sb[:, WGC:]
            ab_sb = []
            off = 0
            for c, njp in enumerate(AB_CH):
                t = wpool.tile([128, njp * BC], BF, tag=f"ab{c}")
                ab_sb.append((t, off))
                off += njp
            sgs = {
                k: wpool.tile([SGR, n * BC], BF, tag=f"sg_{k}", name=f"sg_{k}")
                for k, (_, n) in GRPS.items()
            }
            qts = {
                'A': wpool.tile([KA, I * BC], BF, tag="qtA", name="qtA"),
                'B': wpool.tile([KB, I * BC], BF, tag="qtB", name="qtB"),
            }
            merged = mpool.tile([128, 16], FP, tag="mg")
            scb = mpool.tile([128, P], BF, tag="scb", bufs=2, name="scb")
            ex = mpool.tile([128, 16], FP, tag="ex")
            sm = mpool.tile([128, 4], FP, tag="sm")
            rc = mpool.tile([128, 4], FP, tag="rc")
            pa = mpool.tile([128, 8], FP, tag="pa")
            fin = mpool.tile([128, 16], FP, tag="fin")

            wb_sb = wg_sb[:, 0:24]
            wu_sb = wg_sb[:, 24:36]
            g_sb = {
                'A': wg_sb[0:KA, 36:36 + P],
                'B': wg_sb[0:KB, 36 + P:36 + 2 * P],
            }

            # ---- input DMAs (sync queue: strict ring order) ----
            nc.sync.dma_start(wa_sb[:], wa.ap()[:])
            off = 0
            for c, njp in enumerate(AB_CH):
                nc.sync.dma_start(
                    ab_sb[c][0][:], ab.ap()[:, off * BC:(off + njp) * BC]
                )
                off += njp
            nc.vector.memset(fin[:], 0.0)

            # ---- stage-1 helpers ----
            def bank_mms(g, pb):
                for s, (kind, ident) in enumerate(BANKS[g]):
                    if kind == 'u':
                        lhs = wu_sb
                        dat = au_sb[:, ident * BC:(ident + 1) * BC]
                        m = 12
                    else:
                        lhs = wb_sb
                        pos = AB_POS[ident]
                        for (tl, o0), njp in zip(ab_sb, AB_CH):
                            if pos < o0 + njp:
                                dat = tl[:, (pos - o0) * BC:(pos - o0 + 1) * BC]
                                break
                        m = 24
                    nc.tensor.matmul(
                        pb[32 * s:32 * s + m, :], lhs, dat,
                        start=True, stop=True,
                    )

            def evac(g, sgt, cg):
                pb = psb.tile([SGR, BC], FP, tag="pb")
                bank_mms(g, pb)
                dst = sgt[:, cg * BC:(cg + 1) * BC]
                if g % 2:
                    nc.scalar.activation(
                        dst, pb[:], mybir.ActivationFunctionType.Copy
                    )
                else:
                    nc.vector.tensor_copy(dst, pb[:])

            def hop1(grp, eng):
                ng = GRPS[grp][1]
                dst = scrs[grp].ap()[:, :].rearrange(
                    "g (p b) -> p g b", p=SCRP
                )[0:SGR]
                src = sgs[grp][:, 0:ng * BC].rearrange(
                    "p (g b) -> p g b", g=ng
                )
                return eng.dma_start(dst, src)

            def hop2(spec, h1, eng, split=None):
                chunk, r0, grp, gl, ng, s0, ns, u0, nu = spec
                qtc = qts[chunk]
                tc.dep_state.set_after_insts(qtc.tensor.name, h1.ins)
                src = scrs[grp].ap()[gl:gl + ng, :].rearrange(
                    "g (s u ib) -> g s u ib", s=3, u=8
                )[:, s0:s0 + ns, u0:u0 + nu, :]
                nr = ng * ns * nu
                kw = {"max_dma_last_dim": split} if split else {}
                eng.dma_start(qtc[r0:r0 + nr, :], src, **kw)

            # ---- stage-2 helpers ----
            sc_tiles = {}

            def mmA(t):
                bt, i = t // 4, t % 4
                sc = pss.tile([128, P], FP, tag="sc")
                sc_tiles[t] = sc
                c = i * BC + bt * 128
                nc.tensor.matmul(
                    sc[:], qts['A'][:, c:c + 128], g_sb['A'],
                    start=True, stop=False,
                )

            def softmax(bt):
                # exp on ACT without the accumulator read-back; the sum
                # runs on DVE so the recip chain stays on one engine
                nc.scalar.activation(
                    ex[:, 4 * bt:4 * bt + 4], merged[:, 4 * bt:4 * bt + 4],
                    mybir.ActivationFunctionType.Exp,
                )
                nc.vector.tensor_reduce(
                    sm[:, bt:bt + 1], ex[:, 4 * bt:4 * bt + 4],
                    axis=mybir.AxisListType.X, op=mybir.AluOpType.add,
                )
                nc.vector.reciprocal(rc[:, bt:bt + 1], sm[:, bt:bt + 1])
                e2 = ex[:, 4 * bt:4 * bt + 4].rearrange(
                    "p (a two) -> p a two", two=2
                )
                nc.vector.tensor_add(
                    pa[:, 2 * bt:2 * bt + 2], e2[:, :, 0], e2[:, :, 1]
                )
                nc.vector.tensor_scalar_mul(
                    fin[:, 4 * bt:4 * bt + 2], pa[:, 2 * bt:2 * bt + 2],
                    rc[:, bt:bt + 1],
                )
                nc.sync.dma_start(
                    out.ap()[:, 4 * bt:4 * bt + 4], fin[:, 4 * bt:4 * bt + 4]
                )

            def mmB(t):
                bt, i = t // 4, t % 4
                sc = sc_tiles.pop(t)
                c = i * BC + bt * 128
                nc.tensor.matmul(
                    sc[:], qts['B'][:, c:c + 128], g_sb['B'],
                    start=False, stop=True,
                )
                if t % 2 == 0 or t >= 12:
                    nc.vector.tensor_reduce(
                        merged[:, t:t + 1], sc[:],
                        axis=mybir.AxisListType.X, op=MIN,
                    )
                else:
                    # offload psum read to the ACT engine; bf16 reduce is
                    # ~2x faster on DVE and the rounding is within budget
                    nc.scalar.activation(
                        scb[:], sc[:], mybir.ActivationFunctionType.Copy
                    )
                    nc.vector.tensor_reduce(
                        merged[:, t:t + 1], scb[:],
                        axis=mybir.AxisListType.X, op=MIN,
                    )
                if i == 3:
                    softmax(bt)

            # ---- stage-1 + interleaved stage-2 ----
            for g in range(0, 3):                    # U0,U1,U2
                evac(g, sgs['a0'], g)
            h1a0 = hop1('a0', nc.scalar)
            hop2(HOP2S[0], h1a0, nc.scalar)
            hop2(HOP2S[1], h1a0, nc.scalar)

            for g in range(3, 5):                    # O0,O1
                evac(g, sgs['a1'], g - 3)
            h1a1 = hop1('a1', nc.sync)
            hop2(HOP2S[2], h1a1, nc.sync)

            for g in range(5, 9):                    # O2..O5 (ab2)
                evac(g, sgs['b1'], g - 5)
            h1b1 = hop1('b1', nc.scalar)
            hop2(HOP2S[3], h1b1, nc.scalar)          # O2..O5 rows

            for t in range(6):                       # early chunk-A mms
                mmA(t)

            for g in range(9, 12):                   # O6..O8 (ab3)
                evac(g, sgs['b2'], g - 9)
            h1b2 = hop1('b2', nc.sync)
            hop2(HOP2S[4], h1b2, nc.sync, split=512)  # O6..O8 (idle sync queue)

            # ---- closing ----
            for t in range(16):
                mmB(t)
                if t + 6 < 16:
                    mmA(t + 6)


    nc.compile()
    return nc


def _get_module():
    if "nc" not in _CACHED:
        _CACHED["nc"] = _build_module()
    return _CACHED["nc"]


def _host_inputs(unary_feats, binary_feats, rule_unary, rule_binary):
    import ml_dtypes

    bf16 = ml_dtypes.bfloat16
    uf = np.asarray(unary_feats, dtype=np.float32).astype(bf16)
    bf = np.asarray(binary_feats, dtype=np.float32).astype(bf16)
    ru = np.asarray(rule_unary, dtype=np.float32)
    rb = np.asarray(rule_binary, dtype=np.float32)

    wb, wu = _build_weights(ru, rb)
    ga, gb = _build_g()
    wgm = np.zeros((128, WGC), np.float32)
    wgm[:, 0:24] = wb
    wgm[:, 24:36] = wu
    wgm[0:KA, 36:36 + P] = ga
    wgm[0:KB, 36 + P:36 + 2 * P] = gb
    wgm = wgm.astype(bf16)

    J = np.array([p[0] for p in _PAIRS])
    K = np.array([p[1] for p in _PAIRS])
    ordr = np.array(AB_ORDER)
    dia = np.arange(O)
    in_maps = []
    for c in range(N_CORES):
        bfc = bf[c * BC:(c + 1) * BC]                  # [BC, O, O, E]
        x0 = bfc.transpose(1, 2, 3, 0)                 # [j, k, e, b]
        pair = np.concatenate([x0[J, K], x0[K, J]], axis=1)  # [28, 128, BC]
        abm = np.ascontiguousarray(
            pair[ordr].transpose(1, 0, 2)
        ).reshape(128, NP * BC)
        ufc = uf[c * BC:(c + 1) * BC]
        ut = ufc.transpose(1, 2, 0)
        dg = bfc[:, dia, dia, :].transpose(1, 2, 0)
        aum = np.ascontiguousarray(
            np.concatenate([ut, dg], axis=1).transpose(1, 0, 2)
        ).reshape(128, O * BC)
        in_maps.append({"ab": abm, "wa": np.concatenate([wgm, aum], axis=1)})
    return in_maps


TRACE = False  # set True (e.g. from test.py) to capture an NTFF profile


def kernel(unary_feats, binary_feats, rule_unary, rule_binary):
    from concourse.bass_utils import run_bass_kernel_spmd

    nc = _get_module()
    in_maps = _host_inputs(unary_feats, binary_feats, rule_unary, rule_binary)
    res = run_bass_kernel_spmd(
        nc, in_maps, core_ids=list(range(N_CORES)), trace=TRACE
    )
    _CACHED["last_results"] = res
    outs = []
    for c in range(N_CORES):
        o = res.results[c]["out"]                      # [128, 16]
        outs.append(
            np.ascontiguousarray(
                o.reshape(128, 4, 4).transpose(1, 0, 2)
            ).reshape(BC, 4)
        )
    return np.concatenate(outs, axis=0)


# revision 31
# speedup vs baseline: 1.0389x; 1.0023x over previous
"""Trainium2 Bass kernel for nn_BaseRuleLearner (pipelined compact design).

Math (per batch element b, reference semantics):
  UM[b,i,v,l]      = sum_e U[b,l,e]  * ru[i,v,e]
  BM[b,i,n,m,j,k]  = sum_e Bf[b,j,k,e] * rb[i,n,m,e]
  scores[b,i,p]    = sum_v UM[b,i,v,perm[p,v]]
                   + sum_{n,m} BM[b,i,n,m,perm[p,n],perm[p,m]]
  merged[b,i]      = min_p scores[b,i,p]
  out[b,:]         = softmax_i(merged) @ one_hot([0,0,1,1])

Design (pure data parallel over B across 8 cores, BC=512 b/core):

Stage-1 (36 matmuls, 12 psum banks x 3 tile_position slots {0,32,64}):
  offdiag slot jp: psum rows ud*4+i (24) = wb.T @ ab[:, jp cols]
    (pair-packed columns [Bf j,k ; Bf k,j], weights per (u,d,i))
  unary slot l:    psum rows v*4+i (12) = wu.T @ au[:, l cols]
    (columns [U l ; Bf l,l], weights [ru i,v ; rb i,v,v])
  Banks: U0=[l0..l2] U1=[l3..l5] U2=[l6,l7,jp27] O0..O8=[jp0..jp26].

Assembly through DRAM (i moves from psum rows to qt columns; engine
copies cannot cross partitions; SBUF-source multi-partition-dim DMAs
scramble):  evac casts psum[96,512] -> sg bf16; hop1 (trivial APs)
sg -> scr[g, p*512+b]; hop2 re-views scr affinely as rows (g,slot,sub)
x cols (i,b), skipping pad rows, into compact qt tiles:
  qtA [66, 2048] = U0,U1,U2,O0,O1   qtB [126, 2048] = O2..O8
RAW through DRAM is not dep-tracked -> set_after_insts.  Per-group
scratch tensors (a0,a1,b1,b2) avoid WAW serialization.  Queues: inputs
+ a1/b2 hop chains + per-bt out on sync; a0/b1 hop chains on scalar
(per-hw-engine ring FIFO keeps same-queue order sane; the tail b2
chain rides the sync queue, idle after the input stream).

Stage-2 (32 matmuls): per (bt,i): psum[128,336] = qtA_slice.T @ GA
(start) + qtB_slice.T @ GB (stop); G are 0/1 built to match qt rows.
DVE min-reduce over 336 perms -> merged.  Softmax without
max-subtraction (|logits| < ~10, exp safe in fp32): exp+accum (ACT),
reciprocal, pair-add, scale.  Out [128, 4*4] fp32, host reorders to
[512, 4].

Pipelining: DMA order wa(weights+G+unary), ab1(jp27,jp0-5),
ab2(jp6-17), ab3(jp18-26); evacs alternate vector/scalar; chunk-A hops
run mid-kernel; 6 early chunk-A stage-2 matmuls fill the PE while ab3
streams; closing interleaves mmB/mmA under psum pressure (pss bufs=6,
psb bufs=2 -- NOTE: other psb/pss splits (3/5, 4/4) produced NaNs on
hardware, apparently a latent scheduling race exposed by timing shifts;
keep 2/6).

Min-reduces: tiles 12-15 (the critical tail) and even tiles reduce
direct on DVE from psum; odd tiles 1-11 are copied psum->bf16 by the
ACT engine first then bf16-reduced on DVE (the closing is otherwise
DVE-reduce-bound; the copy path has higher latency so it is kept off
the final tiles). Splitting each evac across both engines was tried
and is SLOWER (op-count overhead beats the width win).

Known profile (50-54us; +-3-4us thermal/throttle variance between
measurement windows): input stream ~7-24us at the ~400GB/s aggregate
cap (6.9MB total incl. 1.96MB hop round-trip), stage-1 trails to
~26-28us (psb rotation; psb/pss splits other than 2/6 NaN on hw),
closing matmuls to ~44us, ~1.3us preamble + ~8-9us fixed NEFF
semaphore-wipe epilogue (measured ~9.4us on a trivial 2-DMA kernel,
unavoidable from kernel structure).
"""

import itertools
import numpy as np

B, O, E = 4096, 8, 64
I, V = 4, 3
P = 336
N_CORES = 8
BC = B // N_CORES            # 512 batch per core
NP = 28
SGR = 88                     # sg rows carried per bank (3 slots, 24 used each)
SCRP = 96                    # scr row pitch (virtual; rows 88-95 never written)
KA, KB = 66, 126             # stage-2 k-chunk rows

_PERM = np.array(list(itertools.permutations(range(O), V)), dtype=np.int32)
_PAIRS = [(j, k) for j in range(O) for k in range(j + 1, O)]
_PIDX = np.full((O, O), -1, np.int32)
for _i, (_j, _k) in enumerate(_PAIRS):
    _PIDX[_j, _k] = _i
_PAIRS3 = [(0, 1), (0, 2), (1, 2)]

AB_ORDER = [27] + list(range(27))            # ab column order (jp27 first)
AB_POS = {jp: idx for idx, jp in enumerate(AB_ORDER)}
AB_CH = [7, 12, 9]                           # ab DMA chunks (jp counts)

BANKS = (
    [[('u', 0), ('u', 1), ('u', 2)],
     [('u', 3), ('u', 4), ('u', 5)],
     [('u', 6), ('u', 7), ('o', 27)]]
    + [[('o', 3 * g + 0), ('o', 3 * g + 1), ('o', 3 * g + 2)] for g in range(9)]
)
NB_A = 5                     # banks 0..4 -> chunk A (U0,U1,U2,O0,O1)

# hop2 geometry: (chunk, r0, g0, ng, s0, ns, u0, nu) — single source of
# truth for both the device DMAs and the host-side G row mapping.
# scratch groups: grp -> (first bank, n banks)
GRPS = {'a0': (0, 3), 'a1': (3, 2), 'b1': (5, 4), 'b2': (9, 3)}
HOP2S = [
    ('A', 0, 'a0', 0, 3, 0, 3, 0, 3),   # U0,U1,U2 all slots, subs 0..2
    ('A', 27, 'a0', 2, 1, 2, 1, 3, 3),  # U2 slot2 (jp27) subs 3..5
    ('A', 30, 'a1', 0, 2, 0, 3, 0, 6),  # O0,O1
    ('B', 0, 'b1', 0, 4, 0, 3, 0, 6),   # O2..O5 (ab2 banks)
    ('B', 72, 'b2', 0, 3, 0, 3, 0, 6),  # O6..O8 (ab3 banks)
]

WGC = 24 + 12 + 2 * P        # wg cols: wb, wu, GA, GB

_CACHED = {}


def _qt_row_index():
    """(kind, ident, sub) -> (which, row) per HOP2S order."""
    idx = {}
    for chunk, r0, grp, gl, ng, s0, ns, u0, nu in HOP2S:
        g0 = GRPS[grp][0] + gl
        r = r0
        for g in range(g0, g0 + ng):
            for s in range(s0, s0 + ns):
                kind, ident = BANKS[g][s]
                for u in range(u0, u0 + nu):
                    idx[(kind, ident, u)] = (chunk, r)
                    r += 1
    return idx


def _build_g():
    idx = _qt_row_index()
    ga = np.zeros((KA, P), np.float32)
    gb = np.zeros((KB, P), np.float32)

    def mark(key, p):
        chunk, r = idx[key]
        (ga if chunk == 'A' else gb)[r, p] = 1.0

    for p in range(P):
        perm = _PERM[p]
        for v in range(V):
            mark(('u', int(perm[v]), v), p)
        for u, (n, m) in enumerate(_PAIRS3):
            x, y = int(perm[n]), int(perm[m])
            j, k = min(x, y), max(x, y)
            d = int(x > y)
            mark(('o', int(_PIDX[j, k]), u * 2 + d), p)
    return ga, gb


def _build_weights(ru, rb):
    wb = np.zeros((128, 24), np.float32)
    for u, (n, m) in enumerate(_PAIRS3):
        for d in range(2):
            for i in range(I):
                col = (u * 2 + d) * 4 + i
                fst, snd = ((n, m), (m, n)) if d == 0 else ((m, n), (n, m))
                wb[0:64, col] = rb[i, fst[0], fst[1], :]
                wb[64:128, col] = rb[i, snd[0], snd[1], :]
    wu = np.zeros((128, 12), np.float32)
    for v in range(V):
        for i in range(I):
            wu[0:64, v * 4 + i] = ru[i, v, :]
            wu[64:128, v * 4 + i] = rb[i, v, v, :]
    return wb, wu


def _build_module():
    import concourse.tile as tile
    from concourse import bacc, mybir

    FP = mybir.dt.float32
    BF = mybir.dt.bfloat16
    MIN = mybir.AluOpType.min
    nc = bacc.Bacc("TRN2", target_bir_lowering=False, debug=False)

    wa = nc.dram_tensor("wa", [128, WGC + O * BC], BF, kind="ExternalInput")
    ab = nc.dram_tensor("ab", [128, NP * BC], BF, kind="ExternalInput")
    out = nc.dram_tensor("out", [128, 16], FP, kind="ExternalOutput")
    scrs = {
        k: nc.dram_tensor(f"scr_{k}", [n, SCRP * BC], BF, kind="Internal")
        for k, (_, n) in GRPS.items()
    }

    with tile.TileContext(nc) as tc:
        with (
            tc.tile_pool(name="wpool", bufs=1) as wpool,
            tc.tile_pool(name="mpool", bufs=1) as mpool,
            tc.tile_pool(name="psb", bufs=2, space="PSUM") as psb,
            tc.tile_pool(name="pss", bufs=6, space="PSUM") as pss,
        ):
            # ---- persistent tiles ----
            wa_sb = wpool.tile([128, WGC + O * BC], BF, tag="wa")
            wg_sb = wa_sb[:, 0:WGC]
            au_sb = wa_sb[:, WGC:]
            ab_sb = []
            off = 0
            for c, njp in enumerate(AB_CH):
                t = wpool.tile([128, njp * BC], BF, tag=f"ab{c}")
                ab_sb.append((t, off))
                off += njp
            sgs = {
                k: wpool.tile([SGR, n * BC], BF, tag=f"sg_{k}", name=f"sg_{k}")
                for k, (_, n) in GRPS.items()
            }
            qts = {
                'A': wpool.tile([KA, I * BC], BF, tag="qtA", name="qtA"),
                'B': wpool.tile([KB, I * BC], BF, tag="qtB", name="qtB"),
            }
            merged = mpool.tile([128, 16], FP, tag="mg")
            scb = mpool.tile([128, P], BF, tag="scb", bufs=2, name="scb")
            ex = mpool.tile([128, 16], FP, tag="ex")
            sm = mpool.tile([128, 4], FP, tag="sm")
            rc = mpool.tile([128, 4], FP, tag="rc")
            pa = mpool.tile([128, 8], FP, tag="pa")
            fin = mpool.tile([128, 16], FP, tag="fin")

            wb_sb = wg_sb[:, 0:24]
            wu_sb = wg_sb[:, 24:36]
            g_sb = {
                'A': wg_sb[0:KA, 36:36 + P],
                'B': wg_sb[0:KB, 36 + P:36 + 2 * P],
            }

            # ---- input DMAs (sync queue: strict ring order) ----
            nc.sync.dma_start(wa_sb[:], wa.ap()[:])
            off = 0
            for c, njp in enumerate(AB_CH):
                nc.sync.dma_start(
                    ab_sb[c][0][:], ab.ap()[:, off * BC:(off + njp) * BC]
                )
                off += njp
            nc.vector.memset(fin[:], 0.0)

            # ---- stage-1 helpers ----
            def bank_mms(g, pb):
                for s, (kind, ident) in enumerate(BANKS[g]):
                    if kind == 'u':
                        lhs = wu_sb
                        dat = au_sb[:, ident * BC:(ident + 1) * BC]
                        m = 12
                    else:
                        lhs = wb_sb
                        pos = AB_POS[ident]
                        for (tl, o0), njp in zip(ab_sb, AB_CH):
                            if pos < o0 + njp:
                                dat = tl[:, (pos - o0) * BC:(pos - o0 + 1) * BC]
                                break
                        m = 24
                    nc.tensor.matmul(
                        pb[32 * s:32 * s + m, :], lhs, dat,
                        start=True, stop=True,
                    )

            def evac(g, sgt, cg):
                pb = psb.tile([SGR, BC], FP, tag="pb")
                bank_mms(g, pb)
                dst = sgt[:, cg * BC:(cg + 1) * BC]
                if g % 2:
                    nc.scalar.activation(
                        dst, pb[:], mybir.ActivationFunctionType.Copy
                    )
                else:
                    nc.vector.tensor_copy(dst, pb[:])

            def hop1(grp, eng, split=None):
                ng = GRPS[grp][1]
                dst = scrs[grp].ap()[:, :].rearrange(
                    "g (p b) -> p g b", p=SCRP
                )[0:SGR]
                src = sgs[grp][:, 0:ng * BC].rearrange(
                    "p (g b) -> p g b", g=ng
                )
                kw = {"max_dma_last_dim": split} if split else {}
                return eng.dma_start(dst, src, **kw)

            def hop2(spec, h1, eng, split=None):
                chunk, r0, grp, gl, ng, s0, ns, u0, nu = spec
                qtc = qts[chunk]
                tc.dep_state.set_after_insts(qtc.tensor.name, h1.ins)
                src = scrs[grp].ap()[gl:gl + ng, :].rearrange(
                    "g (s u ib) -> g s u ib", s=3, u=8
                )[:, s0:s0 + ns, u0:u0 + nu, :]
                nr = ng * ns * nu
                kw = {"max_dma_last_dim": split} if split else {}
                eng.dma_start(qtc[r0:r0 + nr, :], src, **kw)

            # ---- stage-2 helpers ----
            sc_tiles = {}

            def mmA(t):
                bt, i = t // 4, t % 4
                sc = pss.tile([128, P], FP, tag="sc")
                sc_tiles[t] = sc
                c = i * BC + bt * 128
                nc.tensor.matmul(
                    sc[:], qts['A'][:, c:c + 128], g_sb['A'],
                    start=True, stop=False,
                )

            def softmax(bt):
                # exp on ACT without the accumulator read-back; the sum
                # runs on DVE so the recip chain stays on one engine
                nc.scalar.activation(
                    ex[:, 4 * bt:4 * bt + 4], merged[:, 4 * bt:4 * bt + 4],
                    mybir.ActivationFunctionType.Exp,
                )
                nc.vector.tensor_reduce(
                    sm[:, bt:bt + 1], ex[:, 4 * bt:4 * bt + 4],
                    axis=mybir.AxisListType.X, op=mybir.AluOpType.add,
                )
                nc.vector.reciprocal(rc[:, bt:bt + 1], sm[:, bt:bt + 1])
                e2 = ex[:, 4 * bt:4 * bt + 4].rearrange(
                    "p (a two) -> p a two", two=2
                )
                nc.vector.tensor_add(
                    pa[:, 2 * bt:2 * bt + 2], e2[:, :, 0], e2[:, :, 1]
                )
                nc.vector.tensor_scalar_mul(
                    fin[:, 4 * bt:4 * bt + 2], pa[:, 2 * bt:2 * bt + 2],
                    rc[:, bt:bt + 1],
                )
                nc.sync.dma_start(
                    out.ap()[:, 4 * bt:4 * bt + 4], fin[:, 4 * bt:4 * bt + 4]
                )

            def mmB(t):
                bt, i = t // 4, t % 4
                sc = sc_tiles.pop(t)
                c = i * BC + bt * 128
                nc.tensor.matmul(
                    sc[:], qts['B'][:, c:c + 128], g_sb['B'],
                    start=False, stop=True,
                )
                if t % 2 == 0 or t >= 12:
                    nc.vector.tensor_reduce(
                        merged[:, t:t + 1], sc[:],
                        axis=mybir.AxisListType.X, op=MIN,
                    )
                else:
                    # offload psum read to the ACT engine; bf16 reduce is
                    # ~2x faster on DVE and the rounding is within budget
                    nc.scalar.activation(
                        scb[:], sc[:], mybir.ActivationFunctionType.Copy
                    )
                    nc.vector.tensor_reduce(
                        merged[:, t:t + 1], scb[:],
                        axis=mybir.AxisListType.X, op=MIN,
                    )
                if i == 3:
                    softmax(bt)

            # ---- stage-1 + interleaved stage-2 ----
            for g in range(0, 3):                    # U0,U1,U2
                evac(g, sgs['a0'], g)
            h1a0 = hop1('a0', nc.scalar)
            hop2(HOP2S[0], h1a0, nc.scalar)
            hop2(HOP2S[1], h1a0, nc.scalar)

            for g in range(3, 5):                    # O0,O1
                evac(g, sgs['a1'], g - 3)
            h1a1 = hop1('a1', nc.sync)
            hop2(HOP2S[2], h1a1, nc.sync)

            for g in range(5, 9):                    # O2..O5 (ab2)
                evac(g, sgs['b1'], g - 5)
            h1b1 = hop1('b1', nc.scalar, split=256)
            hop2(HOP2S[3], h1b1, nc.scalar, split=512)  # O2..O5 rows

            for t in range(6):                       # early chunk-A mms
                mmA(t)

            for g in range(9, 12):                   # O6..O8 (ab3)
                evac(g, sgs['b2'], g - 9)
            h1b2 = hop1('b2', nc.sync, split=256)
            hop2(HOP2S[4], h1b2, nc.sync, split=512)  # O6..O8 (idle sync queue)

            # ---- closing ----
            for t in range(16):
                mmB(t)
                if t + 6 < 16:
                    mmA(t + 6)


    nc.compile()
    return nc


def _get_module():
    if "nc" not in _CACHED:
        _CACHED["nc"] = _build_module()
    return _CACHED["nc"]


def _host_inputs(unary_feats, binary_feats, rule_unary, rule_binary):
    import ml_dtypes

    bf16 = ml_dtypes.bfloat16
    uf = np.asarray(unary_feats, dtype=np.float32).astype(bf16)
    bf = np.asarray(binary_feats, dtype=np.float32).astype(bf16)
    ru = np.asarray(rule_unary, dtype=np.float32)
    rb = np.asarray(rule_binary, dtype=np.float32)

    wb, wu = _build_weights(ru, rb)
    ga, gb = _build_g()
    wgm = np.zeros((128, WGC), np.float32)
    wgm[:, 0:24] = wb
    wgm[:, 24:36] = wu
    wgm[0:KA, 36:36 + P] = ga
    wgm[0:KB, 36 + P:36 + 2 * P] = gb
    wgm = wgm.astype(bf16)

    J = np.array([p[0] for p in _PAIRS])
    K = np.array([p[1] for p in _PAIRS])
    ordr = np.array(AB_ORDER)
    dia = np.arange(O)
    in_maps = []
    for c in range(N_CORES):
        bfc = bf[c * BC:(c + 1) * BC]                  # [BC, O, O, E]
        x0 = bfc.transpose(1, 2, 3, 0)                 # [j, k, e, b]
        pair = np.concatenate([x0[J, K], x0[K, J]], axis=1)  # [28, 128, BC]
        abm = np.ascontiguousarray(
            pair[ordr].transpose(1, 0, 2)
        ).reshape(128, NP * BC)
        ufc = uf[c * BC:(c + 1) * BC]
        ut = ufc.transpose(1, 2, 0)
        dg = bfc[:, dia, dia, :].transpose(1, 2, 0)
        aum = np.ascontiguousarray(
            np.concatenate([ut, dg], axis=1).transpose(1, 0, 2)
        ).reshape(128, O * BC)
        in_maps.append({"ab": abm, "wa": np.concatenate([wgm, aum], axis=1)})
    return in_maps


TRACE = False  # set True (e.g. from test.py) to capture an NTFF profile


def kernel(unary_feats, binary_feats, rule_unary, rule_binary):
    from concourse.bass_utils import run_bass_kernel_spmd

    nc = _get_module()
    in_maps = _host_inputs(unary_feats, binary_feats, rule_unary, rule_binary)
    res = run_bass_kernel_spmd(
        nc, in_maps, core_ids=list(range(N_CORES)), trace=TRACE
    )
    _CACHED["last_results"] = res
    outs = []
    for c in range(N_CORES):
        o = res.results[c]["out"]                      # [128, 16]
        outs.append(
            np.ascontiguousarray(
                o.reshape(128, 4, 4).transpose(1, 0, 2)
            ).reshape(BC, 4)
        )
    return np.concatenate(outs, axis=0)
